# revision 63
# baseline (speedup 1.0000x reference)
"""Trainium2 Bass kernel for nn_DeformLikeASPPConv (8-core data parallel).

Self-contained: kernel(**inputs) takes the full-batch inputs and returns the
full output. One sample per NeuronCore. See emit() for the device pipeline.

Two-phase pipeline (per core, one sample [64, 256, 256]):
  Phase A (software-pipelined over 64-row blocks): offset-head 3x3 conv via
  the 18-partial trick (all bf16; results packed 3-wide across PSUM
  partition groups {0,32,64} so one engine copy + one unstack DMA moves 3
  rows), then tanh + sampling coords + compound bilinear weights
  (c00..c11) + wrapped-16 i16 gather indices, all in a block-local compact
  [128, 128] layout.
  Phase BC (per 16-row block): dma_gather of the 4 bilinear neighbors
  (one 512B-elem gather via the x_pm4 layout) -> compound-weight combine
  (DVE) -> warped rows into a 96-row ring (partitions 64:128 hold +24-row
  copies so the dilated conv's r+-12 taps pair into K=128 matmuls) ->
  dilated 3x3 conv in 32-row chunks (PE) + BN/ReLU -> bf16 output.
  Index/weight-map fetches ride the Pool (SWDGE) queue to keep the global
  HWDGE queue and SP stream clear.
"""
import sys
if "/opt/trn_rl_repo" not in sys.path:
    sys.path.insert(0, "/opt/trn_rl_repo")
import numpy as np
import ml_dtypes
import concourse.bass as bass
import concourse.bacc as bacc
import concourse.tile as tile
import concourse.mybir as mybir
from concourse import bass_utils

N_CORES = 8
H, W = 256, 256
N = H * W

NPBF16 = ml_dtypes.bfloat16
C = 64
DIL = 12
BN_EPS = 1e-5

RA = 64          # A-block rows
RB = 16          # BC-block rows
MB = RB * W      # 4096 pixels per BC block
RING = 96        # warped ring rows
NA = RA * W      # 8192 pixels per A-block
CA = NA // 128   # 64 compact cols per A-block
NAB = H // RA    # 8 A-blocks
XH = (RA + 2) // 2 + 1  # 18 rows in first xt half


def prep_core_inputs(x, offset_w, offset_b, conv_w, bn_gamma, bn_beta,
                     bn_mean, bn_var):
    """x: [C, H, W] fp32 one sample -> dict of kernel inputs."""
    base = prep_shared(offset_w, offset_b, conv_w, bn_gamma, bn_beta,
                       bn_mean, bn_var)
    base.update(prep_x(x))
    return base


def prep_x(x):
    x_cm = x.reshape(C, N).astype(NPBF16)
    pm = np.ascontiguousarray(x.reshape(C, N).T).astype(NPBF16)  # [N, C]
    p = np.arange(N)
    x_pm4 = np.concatenate([
        pm[np.minimum(p + d, N - 1)] for d in (0, 1, W, W + 1)],
        axis=1)  # [N, 4C]
    return {"x_cm": x_cm, "x_pm4": x_pm4}


def prep_shared(offset_w, offset_b, conv_w, bn_gamma, bn_beta, bn_mean,
                bn_var):
    wo18 = np.zeros((C, 32), np.float32)  # cols 18:32 zero-padded so the
    for t in range(9):                    # packed matmuls fill whole
        r, s = t // 3, t % 3              # 32-partition PSUM groups
        for o in range(2):
            wo18[:, 2 * t + o] = offset_w[o, :, r, s]
    sel18 = np.zeros((18, 32), np.float32)
    for t in range(9):
        for o in range(2):
            sel18[2 * t + o, o] = 1.0
    inv = (bn_gamma / np.sqrt(bn_var + BN_EPS)).astype(np.float32)
    wmf = conv_w * inv[:, None, None, None]  # [Cout, Cin, 3, 3]
    wm1 = np.zeros((C, 3 * C), np.float32)
    wm1a = np.zeros((C, 3 * C), np.float32)
    wm1b = np.zeros((C, 3 * C), np.float32)
    wm2 = np.zeros((2 * C, 3 * C), np.float32)
    for gs in range(3):  # gcol = (ds+1)*C with ds = gs-1
        wm1[:, gs * C:(gs + 1) * C] = wmf[:, :, 1, gs].T
        wm1a[:, gs * C:(gs + 1) * C] = wmf[:, :, 0, gs].T
        wm1b[:, gs * C:(gs + 1) * C] = wmf[:, :, 2, gs].T
        wm2[0:C, gs * C:(gs + 1) * C] = wmf[:, :, 0, gs].T
        wm2[C:2 * C, gs * C:(gs + 1) * C] = wmf[:, :, 2, gs].T
    biasy = (bn_beta - bn_mean * inv).astype(np.float32).reshape(C, 1)
    # block-local compact maps: A-block a, partition p, col c ->
    # global pixel 8192*a + 64*p + c
    parts = np.arange(128)[:, None]
    cols = np.arange(CA)[None, :]
    jm2 = np.zeros((128, NAB * CA), np.float32)
    im2 = np.zeros((128, NAB * CA), np.float32)
    pb2 = np.zeros((128, NAB), np.float32)
    for a in range(NAB):
        g = NA * a + CA * parts + cols
        jm2[:, a * CA:(a + 1) * CA] = g % W
        im2[:, a * CA:(a + 1) * CA] = g // W
        blk = (RA // RB) * a + parts[:, 0] // (128 * RB // RA)
        pb2[:, a] = np.maximum(0, RB * blk - 2) * W
    return {
        "wo18": wo18.astype(NPBF16),
        "sel18": sel18.astype(NPBF16),
        "wm1": wm1.astype(NPBF16),
        "wm1a": wm1a.astype(NPBF16),
        "wm1b": wm1b.astype(NPBF16),
        "wm2": wm2.astype(NPBF16),
        "offbp": np.broadcast_to(offset_b.astype(np.float32)[None, :],
                                 (128, 2)).copy(),
        "biasy": biasy,
        "jmap": jm2,
        "imap": im2,
        "pbase": pb2,
    }


IN_SPECS = [
    ("x_cm", (C, N), NPBF16),
    ("x_pm4", (N, 4 * C), NPBF16),
    ("wo18", (C, 32), NPBF16),
    ("sel18", (18, 32), NPBF16),
    ("wm1", (C, 3 * C), NPBF16),
    ("wm1a", (C, 3 * C), NPBF16),
    ("wm1b", (C, 3 * C), NPBF16),
    ("wm2", (2 * C, 3 * C), NPBF16),
    ("offbp", (128, 2), np.float32),
    ("biasy", (C, 1), np.float32),
    ("jmap", (128, NAB * CA), np.float32),
    ("imap", (128, NAB * CA), np.float32),
    ("pbase", (128, NAB), np.float32),
]

F32 = mybir.dt.float32
BF16 = mybir.dt.bfloat16
I16 = mybir.dt.int16
I32 = mybir.dt.int32
ALU = mybir.AluOpType
AF = mybir.ActivationFunctionType

CLX = (W - 2) + 0.99609375
CLY = (H - 2) + 0.99609375


def emit(tc, io, H_, W_):
    nc = tc.nc
    Po = W + 2

    x_cm, x_pm4 = io["x_cm"], io["x_pm4"]
    wo18, sel18 = io["wo18"], io["sel18"]
    wm1, wm1a, wm1b, wm2 = io["wm1"], io["wm1a"], io["wm1b"], io["wm2"]
    offbp, biasy = io["offbp"], io["biasy"]
    jmap, imap, pbase = io["jmap"], io["imap"], io["pbase"]
    y_out = io["y"]

    with tc.tile_pool(name="dram", bufs=1, space="DRAM") as dramp, \
         tc.tile_pool(name="consts", bufs=1) as cstp:
        # debug builds pass these as ExternalOutputs via io
        ox_dram = io.get("dbg_ox") or dramp.tile([2, N], F32)
        cmaps = io.get("dbg_cm") or dramp.tile([4, N], BF16)
        idxw = io.get("dbg_ix") or dramp.tile([1, N], I16)

        offbp_s = cstp.tile([128, 2], F32, tag="offbp")
        nc.sync.dma_start(offbp_s[:], offbp[:])
        biasy_s = cstp.tile([C, 1], F32, tag="biasy")
        nc.sync.dma_start(biasy_s[:], biasy[:])
        pb2_s = cstp.tile([128, NAB], F32, tag="pbase")
        nc.sync.dma_start(pb2_s[:], pbase[:])
        wo18_s = cstp.tile([128, 32], BF16, tag="wo18")
        nc.sync.dma_start(
            wo18_s[:], bass.AP(tensor=wo18[:].tensor, offset=wo18[:].offset,
                               ap=[[0, 2], [32, C], [1, 32]]))
        sel18_s = cstp.tile([18, 32], BF16, tag="sel18")
        nc.sync.dma_start(sel18_s[:], sel18[:])
        jm2 = cstp.tile([128, NAB * CA], F32, tag="jm2")
        nc.sync.dma_start(jm2[:], jmap[:])
        im2 = cstp.tile([128, NAB * CA], F32, tag="im2")
        nc.sync.dma_start(im2[:], imap[:])
        wm1_s = cstp.tile([C, 3 * C], BF16, tag="wm1")
        nc.sync.dma_start(wm1_s[:], wm1[:])
        wm1a_s = cstp.tile([C, 3 * C], BF16, tag="wm1a")
        nc.sync.dma_start(wm1a_s[:], wm1a[:])
        wm1b_s = cstp.tile([C, 3 * C], BF16, tag="wm1b")
        nc.sync.dma_start(wm1b_s[:], wm1b[:])
        wm2_s = cstp.tile([128, 3 * C], BF16, tag="wm2")
        nc.sync.dma_start(wm2_s[:], wm2[:])

        with tc.tile_pool(name="xa", bufs=2) as xap, \
             tc.tile_pool(name="o18", bufs=1) as o18p, \
             tc.tile_pool(name="al", bufs=1) as alp, \
             tc.tile_pool(name="stg", bufs=2) as stgp, \
             tc.tile_pool(name="oxs", bufs=1) as oxsp, \
             tc.tile_pool(name="mp", bufs=2) as mp, \
             tc.tile_pool(name="w2", bufs=1) as w2p, \
             tc.tile_pool(name="gb", bufs=2) as gbp, \
             tc.tile_pool(name="cwb", bufs=2) as cwp, \
             tc.tile_pool(name="ixb", bufs=2) as ixp, \
             tc.tile_pool(name="yb", bufs=2) as ybp, \
             tc.tile_pool(name="psA", bufs=2, space="PSUM") as psA, \
             tc.tile_pool(name="psA2", bufs=2, space="PSUM") as psA2, \
             tc.tile_pool(name="psC", bufs=2, space="PSUM") as psC:
            W2 = w2p.tile([128, RING * W], BF16, tag="W2")

            cp = {"i": 0}
            albuf = {}

            def pcopy(dst, src):
                if cp["i"] % 2 == 0:
                    nc.vector.tensor_copy(dst, src)
                else:
                    nc.scalar.activation(dst, src, AF.Copy, scale=1.0)
                cp["i"] += 1

            xtbuf = {}

            def load_x(a):
                lo = max(0, RA * a - 1)
                hi = min(H, RA * a + RA + 1)
                nr = hi - lo
                xt = xap.tile([128, XH * W], BF16, tag="xa")
                nc.sync.dma_start(xt[0:C, 0:XH * W],
                                  x_cm[:, lo * W:(lo + XH) * W])
                nc.sync.dma_start(xt[C:2 * C, 0:(nr - XH) * W],
                                  x_cm[:, (lo + XH) * W:hi * W])
                xtbuf[a] = xt

            def do_stage1(a):
                r0 = RA * a
                lo = max(0, r0 - 1)
                hi = min(H, r0 + RA + 1)
                nr = hi - lo
                nt = (nr + 2) // 3
                xt = xtbuf.pop(a)
                o18s = o18p.tile([18, (RA + 2) * Po], BF16, tag="o18")
                o18v = o18s[:].rearrange("p (r w) -> p r w", w=Po)
                nc.vector.memset(o18v[:, :, 0:1], 0.0)
                nc.vector.memset(o18v[:, :, Po - 1:Po], 0.0)
                if r0 == 0:
                    nc.vector.memset(o18v[:, 0:1, :], 0.0)
                if r0 + RA >= H:
                    nc.vector.memset(o18v[:, RA + 1:RA + 2, :], 0.0)
                stgb = stgp.tile([96, nt * W], BF16, tag="stg")
                for ti in range(nt):
                    ps = psA.tile([96, W], F32, tag="psA")
                    nwr = 0
                    for q in range(3):
                        r = lo + q * nt + ti
                        if r >= hi:
                            continue
                        nwr = q + 1
                        hh = 0 if (r - lo) < XH else 1
                        nc.tensor.matmul(
                            ps[32 * q:32 * q + 32, :],
                            wo18_s[64 * hh:64 * hh + 64, :],
                            xt[64 * hh:64 * hh + 64,
                               (r - lo - XH * hh) * W:
                               (r - lo - XH * hh + 1) * W],
                            start=True, stop=True)
                    pcopy(stgb[0:32 * nwr, ti * W:(ti + 1) * W],
                          ps[0:32 * nwr, :])
                for q in range(3):
                    rlo = lo + q * nt
                    rhi = min(hi, rlo + nt)
                    srow = rlo - r0 + 1
                    nc.sync.dma_start(
                        o18v[:, srow:srow + rhi - rlo, 1:W + 1],
                        stgb[32 * q:32 * q + 18, 0:(rhi - rlo) * W])
                al = alp.tile([18, RA * W], BF16, tag="al")
                for t in range(9):
                    dr, ds = t // 3 - 1, t % 3 - 1
                    nc.sync.dma_start(
                        al[2 * t:2 * t + 2, :].rearrange(
                            "p (r w) -> p r w", w=W),
                        o18v[2 * t:2 * t + 2, 1 + dr:1 + dr + RA,
                             1 + ds:1 + ds + W])
                albuf[a] = al

            def do_sel18(a):
                r0 = RA * a
                al = albuf.pop(a)
                nsg = RA * W // 512  # 16 sel18 segments of 512 px
                ntb = (nsg + 2) // 3
                oxb = oxsp.tile([96, ntb * 512], F32, tag="oxs")
                for ti in range(ntb):
                    ps2 = psA2.tile([96, 512], F32, tag="psA2")
                    nwr = 0
                    for q in range(3):
                        sg = q * ntb + ti
                        if sg >= nsg:
                            continue
                        nwr = q + 1
                        nc.tensor.matmul(
                            ps2[32 * q:32 * q + 32, :], sel18_s[:],
                            al[:, sg * 512:(sg + 1) * 512],
                            start=True, stop=True)
                    pcopy(oxb[0:32 * nwr, ti * 512:(ti + 1) * 512],
                          ps2[0:32 * nwr, :])
                for q in range(3):
                    sgl = q * ntb
                    sgh = min(nsg, sgl + ntb)
                    nc.sync.dma_start(
                        bass.AP(tensor=ox_dram[:].tensor,
                                offset=ox_dram[:].offset + r0 * W + sgl * 512,
                                ap=[[N, 2], [1, (sgh - sgl) * 512]]),
                        oxb[32 * q:32 * q + 2, 0:(sgh - sgl) * 512])

            def do_coords(a):
                def coord_chain(row, base_map, clmax):
                    oc = mp.tile([128, CA], F32, tag=f"oc{row}")
                    nc.sync.dma_start(
                        oc[:], bass.AP(tensor=ox_dram[:].tensor,
                                       offset=ox_dram[:].offset + row * N
                                       + a * NA,
                                       ap=[[CA, 128], [1, CA]]))
                    tn = mp.tile([128, CA], F32, tag=f"tn{row}")
                    nc.scalar.activation(tn[:], oc[:], AF.Tanh,
                                         bias=offbp_s[:, row:row + 1],
                                         scale=1.0)
                    ic = mp.tile([128, CA], F32, tag=f"ic{row}")
                    nc.vector.scalar_tensor_tensor(
                        ic[:], tn[:], 2.0, base_map[:, a * CA:(a + 1) * CA],
                        ALU.mult, ALU.add)
                    nc.vector.tensor_scalar(ic[:], ic[:], 0.0, clmax,
                                            ALU.max, ALU.min)
                    i32t = mp.tile([128, CA], I32, tag=f"i32{row}")
                    nc.vector.tensor_copy(i32t[:], ic[:])
                    c0f = mp.tile([128, CA], F32, tag=f"c0f{row}")
                    nc.vector.tensor_copy(c0f[:], i32t[:])
                    wf = mp.tile([128, CA], F32, tag=f"wf{row}")
                    nc.vector.tensor_tensor(wf[:], ic[:], c0f[:],
                                            ALU.subtract)
                    # hw f32->i32 rounds to nearest; correct to floor
                    msk = mp.tile([128, CA], F32, tag=f"msk{row}")
                    nc.vector.tensor_scalar(msk[:], wf[:], 0.0, None,
                                            ALU.is_lt)
                    nc.vector.tensor_tensor(c0f[:], c0f[:], msk[:],
                                            ALU.subtract)
                    nc.vector.tensor_tensor(wf[:], ic[:], c0f[:],
                                            ALU.subtract)
                    return c0f, wf

                x0f, wxf = coord_chain(0, jm2, CLX)
                y0f, wyf = coord_chain(1, im2, CLY)
                for nm, t in (("dbg_x0f", x0f), ("dbg_wxf", wxf),
                              ("dbg_y0f", y0f), ("dbg_wyf", wyf)):
                    if io.get(nm) is not None:
                        nc.sync.dma_start(io[nm][:, a * CA:(a + 1) * CA],
                                          t[:])
                vx0 = mp.tile([128, CA], F32, tag="vx0")
                nc.vector.tensor_scalar(vx0[:], wxf[:], -1.0, 1.0,
                                        ALU.mult, ALU.add)
                vy0 = mp.tile([128, CA], F32, tag="vy0")
                nc.vector.tensor_scalar(vy0[:], wyf[:], -1.0, 1.0,
                                        ALU.mult, ALU.add)
                cmt = mp.tile([128, 4, CA], BF16, tag="cmt")
                nc.vector.tensor_tensor(cmt[:, 0, :], vy0[:], vx0[:],
                                        ALU.mult)
                nc.vector.tensor_tensor(cmt[:, 1, :], vy0[:], wxf[:],
                                        ALU.mult)
                nc.vector.tensor_tensor(cmt[:, 2, :], wyf[:], vx0[:],
                                        ALU.mult)
                nc.vector.tensor_tensor(cmt[:, 3, :], wyf[:], wxf[:],
                                        ALU.mult)
                nc.sync.dma_start(
                    bass.AP(tensor=cmaps[:].tensor,
                            offset=cmaps[:].offset + a * NA,
                            ap=[[CA, 128], [N, 4], [1, CA]]),
                    cmt[:])
                idxf = mp.tile([128, CA], F32, tag="idxf")
                nc.vector.scalar_tensor_tensor(idxf[:], y0f[:], float(W),
                                               x0f[:], ALU.mult, ALU.add)
                nc.vector.tensor_scalar(idxf[:], idxf[:], pb2_s[:, a:a + 1],
                                        None, ALU.subtract)
                # i16 convert + in-partition (a',b)->(b,a') shuffle; BC
                # block k covers partitions [PPB*j, PPB*(j+1)), local pixel
                # m = (p%PPB)*CA + c, c = 16a'+b  ->  dram pos
                # k*4096 + 256*b + (CA//16)*(p%PPB) + a'
                KPA = RA // RB
                PPB = 128 // KPA
                A2 = CA // 16
                iiw = mp.tile([128, CA], I16, tag="iiw")
                nc.vector.tensor_copy(
                    iiw[:].rearrange("p (b a2) -> p b a2", a2=A2),
                    idxf[:].rearrange("p (a2 b) -> p b a2", b=16))
                for j in range(KPA):
                    k = KPA * a + j
                    nc.sync.dma_start(
                        bass.AP(tensor=idxw[:].tensor,
                                offset=idxw[:].offset + k * MB,
                                ap=[[A2, PPB], [256, 16], [1, A2]]),
                        iiw[PPB * j:PPB * (j + 1), :].rearrange(
                            "p (b a2) -> p b a2", a2=A2))

            # software pipeline: stage1(a) | sel18(a-1) | coords(a-2) |
            # BC blocks of A-block a-3 -- each stage's inputs were produced
            # a full iteration earlier, so no in-order engine stream stalls.
            load_x(0)
            for a in range(NAB + 1):
                if a + 1 < NAB:
                    load_x(a + 1)
                if a < NAB:
                    do_stage1(a)
                if a >= 1:
                    do_sel18(a - 1)
                    do_coords(a - 1)
        # ---------------- Phase BC: gather + combine + conv --------------
        with tc.tile_pool(name="w2", bufs=1) as w2p, \
             tc.tile_pool(name="gb", bufs=5) as gbp, \
             tc.tile_pool(name="cwb", bufs=4) as cwp, \
             tc.tile_pool(name="ixb", bufs=4) as ixp, \
             tc.tile_pool(name="yb", bufs=2) as ybp, \
             tc.tile_pool(name="psC", bufs=4, space="PSUM") as psC:
            W2 = w2p.tile([128, RING * W], BF16, tag="W2")

            def conv_rows(rlo, rhi):
                seg = {-1: (DIL, W, -DIL), 0: (0, W, 0),
                       1: (0, W - DIL, DIL)}
                for r8 in range(rlo, rhi, 8):
                    yb = ybp.tile([C, 8 * W], BF16, tag="yb")
                    for half in range(2):
                        ps = psC.tile([C, 4 * W], F32, tag="psC")
                        for r in range(r8 + 4 * half, r8 + 4 * half + 4):
                            po = (r - r8 - 4 * half) * W
                            mms = []
                            for ds in (0, -1, 1):
                                olo, ohi, dsoff = seg[ds]
                                gcol = (ds + 1) * C
                                base = (r % RING) * W
                                mms.append(
                                    (ps[:, po + olo:po + ohi],
                                     wm1_s[:, gcol:gcol + C],
                                     W2[0:64, base + olo + dsoff:
                                        base + ohi + dsoff]))
                                if DIL <= r < H - DIL:
                                    b2 = ((r - DIL) % RING) * W
                                    mms.append(
                                        (ps[:, po + olo:po + ohi],
                                         wm2_s[:, gcol:gcol + C],
                                         W2[:, b2 + olo + dsoff:
                                            b2 + ohi + dsoff]))
                                elif r < DIL:
                                    b2 = ((r + DIL) % RING) * W
                                    mms.append(
                                        (ps[:, po + olo:po + ohi],
                                         wm1b_s[:, gcol:gcol + C],
                                         W2[0:64, b2 + olo + dsoff:
                                            b2 + ohi + dsoff]))
                                else:
                                    b2 = ((r - DIL) % RING) * W
                                    mms.append(
                                        (ps[:, po + olo:po + ohi],
                                         wm1a_s[:, gcol:gcol + C],
                                         W2[0:64, b2 + olo + dsoff:
                                            b2 + ohi + dsoff]))
                            for mi, (o, l, rr) in enumerate(mms):
                                nc.tensor.matmul(o, l, rr, start=(mi == 0),
                                                 stop=(mi == len(mms) - 1))
                        nc.scalar.activation(
                            yb[:, half * 4 * W:(half + 1) * 4 * W], ps[:],
                            AF.Relu, bias=biasy_s[:], scale=1.0)
                    nc.sync.dma_start(y_out[:, r8 * W:(r8 + 8) * W], yb[:])

            gbuf = {}

            def emit_bc_fetch(k):
                rb0 = k * RB
                base_px = max(0, rb0 - 2) * W
                ixt = ixp.tile([128, MB // 16], I16, tag="ix")
                nc.sync.dma_start(
                    ixt[:], bass.AP(tensor=idxw[:].tensor,
                                    offset=idxw[:].offset + k * MB,
                                    ap=[[0, 8], [MB // 16, 16],
                                        [1, MB // 16]]))
                g = gbp.tile([128, 2, MB], BF16, tag="g")
                nc.gpsimd.dma_gather(
                    g[:], bass.AP(tensor=x_pm4[:].tensor,
                                  offset=x_pm4[:].offset + base_px * 4 * C,
                                  ap=[[4 * C, N - base_px], [1, 4 * C]]),
                    ixt[:], MB, MB, 4 * C, transpose=True,
                    single_packet=False)
                cw = cwp.tile([128, 2, MB], BF16, tag="cw")
                for gi in range(2):
                    nc.sync.dma_start(
                        cw[:, gi, :],
                        bass.AP(tensor=cmaps[:].tensor,
                                offset=cmaps[:].offset + 2 * gi * N
                                + rb0 * W,
                                ap=[[N, 2], [0, 64], [1, MB]]))
                gbuf[k] = (g, cw)

            def emit_bc_compute(k):
                rb0 = k * RB
                g, cw = gbuf.pop(k)
                g0 = g[:, 0, :]
                g1 = g[:, 1, :]
                nc.vector.tensor_tensor(g0, g0, cw[:, 0, :], ALU.mult)
                nc.vector.tensor_tensor(g1, g1, cw[:, 1, :], ALU.mult)
                nc.vector.tensor_tensor(g0, g0, g1, ALU.add)
                slot = (rb0 % RING) * W
                th = g[0:64, 1, :]  # g1 is dead after the add; reuse as the
                nc.vector.tensor_copy(th, g0[64:128])  # base-shift staging
                nc.vector.tensor_tensor(
                    W2[0:64, slot:slot + MB], g0[0:64], th, ALU.add)
                # fill partitions 64:128 (row +24 copies) for slot-rows
                # [rb0-24, rb0-8) in two 8-row pieces
                for s in (rb0 - 24, rb0 - 16):
                    if s < 0:
                        continue
                    dsl = (s % RING) * W
                    ssl = ((s + 24) % RING) * W
                    nc.vector.tensor_copy(
                        W2[64:128, dsl:dsl + 8 * W],
                        W2[0:64, ssl:ssl + 8 * W])
                # conv chunks: small lag-1 chunks during the BC ramp (PE is
                # idle waiting for the first combines anyway), then 32-row
                # chunks one block behind the combines
                if 1 <= k <= 6:
                    conv_rows(16 * (k - 1), 16 * k)
                elif k in (9, 11, 13, 15):
                    conv_rows(16 * k - 48, 16 * k - 16)


            for k in range(N // MB):
                emit_bc_fetch(k)
                emit_bc_compute(k)
            conv_rows(H - 32, H)


_NC_CACHE = {}


def build_io(nc):
    io = {}
    for name, shape, dt in IN_SPECS:
        mdt = BF16 if dt is NPBF16 else F32
        io[name] = nc.dram_tensor(name, list(shape), mdt,
                                  kind="ExternalInput").ap()
    io["y"] = nc.dram_tensor("y", [C, N], BF16, kind="ExternalOutput").ap()
    return io


def build_nc():
    if "nc" in _NC_CACHE:
        return _NC_CACHE["nc"]
    nc = bacc.Bacc("TRN2", target_bir_lowering=False, debug=False,
                   num_devices=N_CORES)
    io = build_io(nc)
    with tile.TileContext(nc) as tc:
        emit(tc, io, H, W)
    nc.compile()
    _NC_CACHE["nc"] = nc
    return nc


def kernel(x, offset_w, offset_b, conv_w, bn_gamma, bn_beta, bn_mean, bn_var):
    x = np.asarray(x, np.float32)
    offset_w = np.asarray(offset_w, np.float32)
    offset_b = np.asarray(offset_b, np.float32)
    conv_w = np.asarray(conv_w, np.float32)
    bn_gamma = np.asarray(bn_gamma, np.float32)
    bn_beta = np.asarray(bn_beta, np.float32)
    bn_mean = np.asarray(bn_mean, np.float32)
    bn_var = np.asarray(bn_var, np.float32)
    B = x.shape[0]
    nc = build_nc()
    shared = prep_shared(offset_w, offset_b, conv_w, bn_gamma, bn_beta,
                         bn_mean, bn_var)
    in_maps = []
    for b in range(B):
        m = dict(shared)
        m.update(prep_x(x[b]))
        in_maps.append(m)
    res = bass_utils.run_bass_kernel_spmd(nc, in_maps,
                                          core_ids=list(range(B)))
    out = np.stack([
        np.asarray(res.results[b]["y"], dtype=np.float32).reshape(C, H, W)
        for b in range(B)])
    return out
